# revision 16
# baseline (speedup 1.0000x reference)
"""8x8 block DCT (DCT-II) on [64,1,1024,1024] fp32 -> [64,64,128,128].

Data parallel over batch: 8 images per NeuronCore on 8 cores.

Single-stage Kronecker fp16 pipeline (harness gate rel_err < 2e-2;
measured ~6e-4): the 2D DCT of one 8x8 block is a single 64x64 matmul
  coeffs[8u+v] = K64[:, 8u+v] . block_vec,  K64[8x+y, 8u+v] = M[u,x]M[v,y]
so each image is one [64,64]^T @ [64,16384] GEMM. Host casts to fp16 and
pre-permutes blocks into the SBUF layout (pure data movement; all FLOPs
on device); host un-permutes + casts the fp16 result back to fp32.

PE array packing: the K=64 contraction only uses half the array, so two
independent 64x64 tiles run at tile_position (0,0) and (64,64) - blocks
half A on SBUF partitions 0-63, half B on partitions 64-127. Each PSUM
tile [128, 512] holds two 64-row results and drains with ONE full-width
DVE/ACT copy (fp32->fp16). Every output element crosses PSUM->SBUF once
(the two-stage separable form crosses twice), halving vector/scalar work.

All DMAs are fully contiguous 2 MB/image transfers chunked for overlap;
per-core HBM traffic is 16 MB in + 16 MB out -> ~90 us roofline, plus
~8.5 us fixed runtime preamble.

x_dram[img, 64*half + 8x+y, (hb%64)*128 + wb] = x[img, 0, 8hb+x, 8wb+y]
  with half = hb//64
out_dram[img, 64*half + 8u+v, 512k + j]: coeff (u,v) of block
  blkid = half*8192 + 512k + j   (blkid = hb*128 + wb)
"""

import numpy as np

_N_CORES = 8
_H = 1024
_W = 1024

_NC_CACHE = {}

# tuning knobs
IN_ENGINE = "s"  # DMA descriptor-gen path: s=sync(HWDGE), c=scalar, g=gpsimd
OUT_ENGINES = "g"
DRAIN_ENGINES = "vc"  # PSUM->SBUF fp32->fp16 drains (v=vector, c=scalar)
ZIMG_BUFS = 3
XS_BUFS = 3
PS_BUFS = 4
CHUNK = 512  # moving columns per matmul (one PSUM bank)


def _dct_mat_np():
    n = 8
    u = np.arange(n)[:, None].astype(np.float64)
    x = np.arange(n)[None, :].astype(np.float64)
    m = np.cos((2 * x + 1) * u * np.pi / (2 * n))
    scale = np.where(u == 0, np.sqrt(1.0 / n), np.sqrt(2.0 / n))
    return (m * scale).astype(np.float32)


def _build_k64(dct: np.ndarray) -> np.ndarray:
    """K64[8x+y, 8u+v] = dct[u,x] * dct[v,y]."""
    k = np.einsum("ux,vy->xyuv", dct.astype(np.float64), dct.astype(np.float64))
    return k.reshape(64, 64).astype(np.float32)


def build_nc(
    n_img: int,
    in_engine=IN_ENGINE,
    out_engines=OUT_ENGINES,
    drain_engines=DRAIN_ENGINES,
    zimg_bufs=ZIMG_BUFS,
    xs_bufs=XS_BUFS,
    ps_bufs=PS_BUFS,
):
    import concourse.bacc as bacc
    import concourse.mybir as mybir
    import concourse.tile as tile

    f16 = mybir.dt.float16
    f32 = mybir.dt.float32
    nc = bacc.Bacc("TRN2", target_bir_lowering=False, debug=False)

    x = nc.dram_tensor("x", [n_img, 128, 8 * _W], f16, kind="ExternalInput")
    k64 = nc.dram_tensor("k64", [64, 64], f16, kind="ExternalInput")
    out = nc.dram_tensor("out", [n_img, 128, 8 * _W], f16, kind="ExternalOutput")

    def eng(ch):
        return {"s": nc.sync, "c": nc.scalar, "g": nc.gpsimd, "v": nc.vector}[ch]

    def copy_on(ch, dst, src):
        if ch == "v":
            nc.vector.tensor_copy(dst, src)
        else:
            nc.scalar.copy(dst, src)

    n_chunks = (8 * _W) // CHUNK  # 16
    n_out_dma = 0
    n_drain = 0

    with tile.TileContext(nc) as tc:
        with (
            tc.tile_pool(name="const", bufs=1) as constp,
            tc.tile_pool(name="xs", bufs=xs_bufs) as xsp,
            tc.tile_pool(name="zimg", bufs=zimg_bufs) as zp,
            tc.tile_pool(name="ps", bufs=ps_bufs, space="PSUM") as psp,
        ):
            # K64 replicated on both partition halves for (0,0)/(64,64) tiles
            k_t = constp.tile([128, 64], f16)
            nc.scalar.dma_start(k_t[0:64, :], k64[:])
            nc.scalar.dma_start(k_t[64:128, :], k64[:])

            for img in range(n_img):
                # split loads/stores so compute overlaps at sub-image grain
                n_in = 8 if img == 0 else 4
                n_out = 8 if img == n_img - 1 else 4
                xs = xsp.tile([128, 8 * _W], f16)
                for kk in range(n_in):
                    lo, hi = kk * (8 * _W) // n_in, (kk + 1) * (8 * _W) // n_in
                    eng(in_engine).dma_start(xs[:, lo:hi], x[img, :, lo:hi])

                zimg = zp.tile([128, 8 * _W], f16)

                per_out = n_chunks // n_out
                for k in range(n_chunks):
                    lo, hi = k * CHUNK, (k + 1) * CHUNK
                    z_ps = psp.tile([128, CHUNK], f32)
                    nc.tensor.matmul(
                        z_ps[0:64, :], k_t[0:64, :], xs[0:64, lo:hi],
                        start=True, stop=True,
                    )
                    nc.tensor.matmul(
                        z_ps[64:128, :], k_t[64:128, :], xs[64:128, lo:hi],
                        start=True, stop=True,
                    )
                    copy_on(
                        drain_engines[n_drain % len(drain_engines)],
                        zimg[:, lo:hi],
                        z_ps[:],
                    )
                    n_drain += 1

                    # ship completed zimg ranges while later chunks compute
                    if (k + 1) % per_out == 0:
                        olo = (k + 1 - per_out) * CHUNK
                        ohi = (k + 1) * CHUNK
                        if img == n_img - 1:
                            # tail: sync HWDGE ring (idle by now), avoids the
                            # slow SWDGE engine-15 straggler on final stores
                            e = "s"
                        else:
                            e = out_engines[n_out_dma % len(out_engines)]
                        n_out_dma += 1
                        eng(e).dma_start(out[img, :, olo:ohi], zimg[:, olo:ohi])

    nc.compile()
    return nc


def _get_nc(n_img: int):
    if n_img not in _NC_CACHE:
        _NC_CACHE[n_img] = build_nc(n_img)
    return _NC_CACHE[n_img]


def _prep_x(x_core: np.ndarray) -> np.ndarray:
    """[per,1,1024,1024] fp32 -> [per,128,8192] fp16 block layout."""
    per = x_core.shape[0]
    x16 = x_core.astype(np.float16)
    # [per, (half,hb2,x), (wb,y)] -> [per, (half, x, y), (hb2, wb)]
    xp = (
        x16.reshape(per, 2, 64, 8, 128, 8)
        .transpose(0, 1, 3, 5, 2, 4)
        .reshape(per, 128, 8 * _W)
    )
    return np.ascontiguousarray(xp)


def _unprep_out(od: np.ndarray) -> np.ndarray:
    """[per,128,8192] fp16 raw dump -> [per,64,128,128] fp32."""
    per = od.shape[0]
    # od[img, (half, ch), blk2] ; blkid = half*8192 + blk2
    o = (
        od.reshape(per, 2, 64, 8 * _W)
        .transpose(0, 2, 1, 3)
        .reshape(per, 64, 128, 128)
    )
    return o.astype(np.float32)


def make_inputs(x_core: np.ndarray, dct: np.ndarray) -> dict:
    return {"x": _prep_x(x_core), "k64": _build_k64(dct).astype(np.float16)}


def run_spmd(x: np.ndarray, dct: np.ndarray, trace: bool = False, nc=None):
    """Run the SPMD kernel on 8 cores. Returns (out, BassKernelResults)."""
    from concourse.bass_utils import run_bass_kernel_spmd

    x = np.ascontiguousarray(np.asarray(x, dtype=np.float32))
    dct = np.asarray(dct, dtype=np.float32)
    b = x.shape[0]
    per = b // _N_CORES

    if nc is None:
        nc = _get_nc(per)
    in_maps = [
        make_inputs(x[i * per : (i + 1) * per], dct) for i in range(_N_CORES)
    ]
    res = run_bass_kernel_spmd(
        nc, in_maps, core_ids=list(range(_N_CORES)), trace=trace
    )
    out = np.concatenate(
        [_unprep_out(res.results[i]["out"]) for i in range(_N_CORES)], axis=0
    )
    return out, res


def kernel(x, dct=None):
    if dct is None:
        dct = _dct_mat_np()
    out, _ = run_spmd(x, dct, trace=False)
    return out


# revision 17
# speedup vs baseline: 1.1087x; 1.1087x over previous
"""8x8 block DCT (DCT-II) on [64,1,1024,1024] fp32 -> [64,64,128,128].

Data parallel over batch: 8 images per NeuronCore on 8 cores.

Single-stage Kronecker fp16 pipeline (harness gate rel_err < 2e-2;
measured ~6e-4): the 2D DCT of one 8x8 block is a single 64x64 matmul
  coeffs[8u+v] = K64[:, 8u+v] . block_vec,  K64[8x+y, 8u+v] = M[u,x]M[v,y]
so each image is one [64,64]^T @ [64,16384] GEMM. Host casts to fp16 and
pre-permutes blocks into the SBUF layout (pure data movement; all FLOPs
on device); host un-permutes + casts the fp16 result back to fp32.

PE array packing: the K=64 contraction only uses half the array, so two
independent 64x64 tiles run at tile_position (0,0) and (64,64) - blocks
half A on SBUF partitions 0-63, half B on partitions 64-127. Each PSUM
tile [128, 512] holds two 64-row results and drains with ONE full-width
DVE/ACT copy (fp32->fp16). Every output element crosses PSUM->SBUF once
(the two-stage separable form crosses twice), halving vector/scalar work.

All DMAs are fully contiguous 2 MB/image transfers chunked for overlap;
per-core HBM traffic is 16 MB in + 16 MB out -> ~90 us roofline, plus
~8.5 us fixed runtime preamble.

x_dram[img, 64*half + 8x+y, (hb%64)*128 + wb] = x[img, 0, 8hb+x, 8wb+y]
  with half = hb//64
out_dram[img, 64*half + 8u+v, 512k + j]: coeff (u,v) of block
  blkid = half*8192 + 512k + j   (blkid = hb*128 + wb)
"""

import numpy as np

_N_CORES = 8
_H = 1024
_W = 1024

_NC_CACHE = {}

# tuning knobs
IN_ENGINE = "g"  # DMA descriptor-gen path: s=sync(HWDGE), c=scalar, g=gpsimd
OUT_ENGINES = "s"
DRAIN_ENGINES = "vc"  # PSUM->SBUF fp32->fp16 drains (v=vector, c=scalar)
ZIMG_BUFS = 3
XS_BUFS = 3
PS_BUFS = 4
CHUNK = 512  # moving columns per matmul (one PSUM bank)


def _dct_mat_np():
    n = 8
    u = np.arange(n)[:, None].astype(np.float64)
    x = np.arange(n)[None, :].astype(np.float64)
    m = np.cos((2 * x + 1) * u * np.pi / (2 * n))
    scale = np.where(u == 0, np.sqrt(1.0 / n), np.sqrt(2.0 / n))
    return (m * scale).astype(np.float32)


def _build_k64(dct: np.ndarray) -> np.ndarray:
    """K64[8x+y, 8u+v] = dct[u,x] * dct[v,y]."""
    k = np.einsum("ux,vy->xyuv", dct.astype(np.float64), dct.astype(np.float64))
    return k.reshape(64, 64).astype(np.float32)


def build_nc(
    n_img: int,
    in_engine=IN_ENGINE,
    out_engines=OUT_ENGINES,
    drain_engines=DRAIN_ENGINES,
    zimg_bufs=ZIMG_BUFS,
    xs_bufs=XS_BUFS,
    ps_bufs=PS_BUFS,
):
    import concourse.bacc as bacc
    import concourse.mybir as mybir
    import concourse.tile as tile

    f16 = mybir.dt.float16
    f32 = mybir.dt.float32
    nc = bacc.Bacc("TRN2", target_bir_lowering=False, debug=False)

    x = nc.dram_tensor("x", [n_img, 128, 8 * _W], f16, kind="ExternalInput")
    k64 = nc.dram_tensor("k64", [64, 64], f16, kind="ExternalInput")
    out = nc.dram_tensor("out", [n_img, 128, 8 * _W], f16, kind="ExternalOutput")

    def eng(ch):
        return {"s": nc.sync, "c": nc.scalar, "g": nc.gpsimd, "v": nc.vector}[ch]

    def copy_on(ch, dst, src):
        if ch == "v":
            nc.vector.tensor_copy(dst, src)
        else:
            nc.scalar.copy(dst, src)

    n_chunks = (8 * _W) // CHUNK  # 16
    n_out_dma = 0
    n_drain = 0

    with tile.TileContext(nc) as tc:
        with (
            tc.tile_pool(name="const", bufs=1) as constp,
            tc.tile_pool(name="xs", bufs=xs_bufs) as xsp,
            tc.tile_pool(name="zimg", bufs=zimg_bufs) as zp,
            tc.tile_pool(name="ps", bufs=ps_bufs, space="PSUM") as psp,
        ):
            # K64 replicated on both partition halves for (0,0)/(64,64) tiles
            k_t = constp.tile([128, 64], f16)
            nc.scalar.dma_start(k_t[0:64, :], k64[:])
            nc.scalar.dma_start(k_t[64:128, :], k64[:])

            for img in range(n_img):
                # split loads/stores so compute overlaps at sub-image grain
                n_in = 8 if img == 0 else 4
                n_out = 8 if img == n_img - 1 else 4
                xs = xsp.tile([128, 8 * _W], f16)
                for kk in range(n_in):
                    lo, hi = kk * (8 * _W) // n_in, (kk + 1) * (8 * _W) // n_in
                    eng(in_engine).dma_start(xs[:, lo:hi], x[img, :, lo:hi])

                zimg = zp.tile([128, 8 * _W], f16)

                per_out = n_chunks // n_out
                for k in range(n_chunks):
                    lo, hi = k * CHUNK, (k + 1) * CHUNK
                    z_ps = psp.tile([128, CHUNK], f32)
                    nc.tensor.matmul(
                        z_ps[0:64, :], k_t[0:64, :], xs[0:64, lo:hi],
                        start=True, stop=True,
                    )
                    nc.tensor.matmul(
                        z_ps[64:128, :], k_t[64:128, :], xs[64:128, lo:hi],
                        start=True, stop=True,
                    )
                    copy_on(
                        drain_engines[n_drain % len(drain_engines)],
                        zimg[:, lo:hi],
                        z_ps[:],
                    )
                    n_drain += 1

                    # ship completed zimg ranges while later chunks compute
                    if (k + 1) % per_out == 0:
                        olo = (k + 1 - per_out) * CHUNK
                        ohi = (k + 1) * CHUNK
                        if img == n_img - 1:
                            # tail: sync HWDGE ring (idle by now), avoids the
                            # slow SWDGE engine-15 straggler on final stores
                            e = "s"
                        else:
                            e = out_engines[n_out_dma % len(out_engines)]
                        n_out_dma += 1
                        eng(e).dma_start(out[img, :, olo:ohi], zimg[:, olo:ohi])

    nc.compile()
    return nc


def _get_nc(n_img: int):
    if n_img not in _NC_CACHE:
        _NC_CACHE[n_img] = build_nc(n_img)
    return _NC_CACHE[n_img]


def _prep_x(x_core: np.ndarray) -> np.ndarray:
    """[per,1,1024,1024] fp32 -> [per,128,8192] fp16 block layout."""
    per = x_core.shape[0]
    x16 = x_core.astype(np.float16)
    # [per, (half,hb2,x), (wb,y)] -> [per, (half, x, y), (hb2, wb)]
    xp = (
        x16.reshape(per, 2, 64, 8, 128, 8)
        .transpose(0, 1, 3, 5, 2, 4)
        .reshape(per, 128, 8 * _W)
    )
    return np.ascontiguousarray(xp)


def _unprep_out(od: np.ndarray) -> np.ndarray:
    """[per,128,8192] fp16 raw dump -> [per,64,128,128] fp32."""
    per = od.shape[0]
    # od[img, (half, ch), blk2] ; blkid = half*8192 + blk2
    o = (
        od.reshape(per, 2, 64, 8 * _W)
        .transpose(0, 2, 1, 3)
        .reshape(per, 64, 128, 128)
    )
    return o.astype(np.float32)


def make_inputs(x_core: np.ndarray, dct: np.ndarray) -> dict:
    return {"x": _prep_x(x_core), "k64": _build_k64(dct).astype(np.float16)}


def run_spmd(x: np.ndarray, dct: np.ndarray, trace: bool = False, nc=None):
    """Run the SPMD kernel on 8 cores. Returns (out, BassKernelResults)."""
    from concourse.bass_utils import run_bass_kernel_spmd

    x = np.ascontiguousarray(np.asarray(x, dtype=np.float32))
    dct = np.asarray(dct, dtype=np.float32)
    b = x.shape[0]
    per = b // _N_CORES

    if nc is None:
        nc = _get_nc(per)
    in_maps = [
        make_inputs(x[i * per : (i + 1) * per], dct) for i in range(_N_CORES)
    ]
    res = run_bass_kernel_spmd(
        nc, in_maps, core_ids=list(range(_N_CORES)), trace=trace
    )
    out = np.concatenate(
        [_unprep_out(res.results[i]["out"]) for i in range(_N_CORES)], axis=0
    )
    return out, res


def kernel(x, dct=None):
    if dct is None:
        dct = _dct_mat_np()
    out, _ = run_spmd(x, dct, trace=False)
    return out
